# revision 1
# baseline (speedup 1.0000x reference)
"""Trainium2 Bass kernel for nn_MoDBlock (mixture-of-depths block), v3.

Full inputs in, full outputs out. Sharding: data-parallel over batch B=2
(cores 0-3 -> batch 0, cores 4-7 -> batch 1); within a batch group of 4
cores, attention queries + MLP rows are sharded by INTERLEAVED query tiles
{j, 7-j, 8+j, 15-j} (load-balances the causal triangle so each core runs
the same 40-of-64 key-tile schedule); K/V + router/top-k replicated.

Per-core program (single SPMD NEFF):
  1. router scores s = x @ w_router streamed in 512-token chunks
     (DVE fused mul-reduce, fp32)
  2. exact top-C threshold via 5 levels of 128-ary counting bisection
  3. mask -> ranks (ones-matmul column sums + log-shift prefix + triangular
     matmul within-column prefix) -> ON-CHIP splice: per source column f a
     placement matmul Comb_f^T @ (jmatch_f * ids_f) accumulates the sorted
     selected token ids directly into a [128,16] PSUM tile (no DRAM scatter)
  4. own-row indices via transpose + one-hot matmul over the qts input;
     indirect-gather 16 selected-token tiles + own 4 row tiles; rmsnorm;
     PE-transpose
  5. K^T [1024,2048], V(+ones per head), Q^T own [1024,512]
  6. attention per head over the slot schedule: key tile kt serves query
     slots >= kt//4 (contiguous columns), exp on ACT, one data-dependent
     mask per kt on the leading 128 columns, P^T V accumulated into a
     [65,512] PSUM tile; normalize via ones-column denominator broadcast +
     replicated reciprocal
  7. wo + residual + rmsnorm -> z^T
  8. SwiGLU row-local: gate/up (silu), X^T, down-projection, y = res + mlp
Host: scatters computed rows back into a copy of x using the device idx.
"""
import sys

if '/opt/trn_rl_repo' not in sys.path:
    sys.path.insert(0, '/opt/trn_rl_repo')

import numpy as np
import ml_dtypes

import concourse.bass as bass
import concourse.bacc as bacc
import concourse.mybir as mybir
from concourse.tile import TileContext
from concourse.masks import make_identity
from concourse.bass_utils import run_bass_kernel_spmd

B, T, D = 2, 4096, 1024
H, HD = 16, 64
C = 2048
FF = 4096
P = 128
N_CORES = 8
GROUP = 4
QCH = C // GROUP           # 512 rows owned per core (queries/MLP/residual)

F32 = mybir.dt.float32
BF16 = mybir.dt.bfloat16
I32 = mybir.dt.int32
AX = mybir.AxisListType
OP = mybir.AluOpType
AF = mybir.ActivationFunctionType

NT = T // P      # 32 token tiles of full x
NS = C // P      # 16 token tiles of selected seq
NQ = QCH // P    # 4 query slots owned per core
ND = D // P      # 8 dim tiles
NFF = FF // P    # 32 ff tiles


def slot_tiles(j):
    return [j, 7 - j, 8 + j, 15 - j]


def build_nc():
    nc = bacc.Bacc("TRN2", target_bir_lowering=False, debug=False,
                   enable_asserts=False, num_devices=N_CORES)
    aps = {}
    aps["x"] = nc.dram_tensor("x", (T, D), F32, kind="ExternalInput").ap()
    aps["wr"] = nc.dram_tensor("wr", (1, D), F32, kind="ExternalInput").ap()
    aps["qts"] = nc.dram_tensor("qts", (1, NQ), F32, kind="ExternalInput").ap()
    for n, shp in [("wq", (D, D)), ("wk", (D, D)), ("wv", (D, D)),
                   ("wo", (D, D)), ("wg", (D, FF)), ("wu", (D, FF)),
                   ("wd", (FF, D))]:
        aps[n] = nc.dram_tensor(n, shp, BF16, kind="ExternalInput").ap()
    aps["y"] = nc.dram_tensor("y", (QCH, D), F32, kind="ExternalOutput").ap()
    aps["idx"] = nc.dram_tensor("idx", (C, 1), I32, kind="ExternalOutput").ap()

    with TileContext(nc) as tc:
        _emit(nc, tc, aps)
    nc.compile()
    return nc


def _emit(nc, tc, aps):
    from contextlib import ExitStack
    x, wr, qts = aps["x"], aps["wr"], aps["qts"]
    y_out, idx_out = aps["y"], aps["idx"]

    ctx = ExitStack()
    with ctx:
        const = ctx.enter_context(tc.tile_pool(name="const", bufs=1))

        # ---- constants ----
        ident_f = const.tile([P, P], F32)
        make_identity(nc, ident_f[:])
        ident_b = const.tile([P, P], BF16)
        make_identity(nc, ident_b[:])
        ones_pp = const.tile([P, P], F32)
        nc.vector.memset(ones_pp[:], 1.0)
        ones_row = const.tile([1, P], F32)
        nc.vector.memset(ones_row[:], 1.0)
        iota_p_i = const.tile([P, 1], I32)
        nc.gpsimd.iota(iota_p_i[:], pattern=[[0, 1]], base=0, channel_multiplier=1)
        iota_p = const.tile([P, 1], F32)
        nc.vector.tensor_copy(iota_p[:], iota_p_i[:])
        iota_p1 = const.tile([P, 1], F32)
        nc.vector.tensor_scalar_add(iota_p1[:], iota_p[:], 1.0)
        iota_fp_i = const.tile([P, P], I32)
        nc.gpsimd.iota(iota_fp_i[:], pattern=[[1, P]], base=0, channel_multiplier=0)
        iota_fp = const.tile([P, P], F32)
        nc.vector.tensor_copy(iota_fp[:], iota_fp_i[:])
        tri = const.tile([P, P], F32)  # tri[p, m] = 1 if m > p
        nc.vector.tensor_scalar(tri[:], iota_fp[:], iota_p[:], None, op0=OP.is_gt)
        iota_tok_i = const.tile([P, NT], I32)  # token id 128 f + p
        nc.gpsimd.iota(iota_tok_i[:], pattern=[[P, NT]], base=0, channel_multiplier=1)
        iota_tok_f = const.tile([P, NT], F32)
        nc.vector.tensor_copy(iota_tok_f[:], iota_tok_i[:])
        iota16_i = const.tile([P, NS], I32)   # column index j (same all rows)
        nc.gpsimd.iota(iota16_i[:], pattern=[[1, NS]], base=0, channel_multiplier=0)
        iota16 = const.tile([P, NS], F32)
        nc.vector.tensor_copy(iota16[:], iota16_i[:])
        fmp = const.tile([P, P], F32)  # q - k within a 128 tile
        nc.vector.tensor_scalar(fmp[:], iota_fp[:], iota_p[:], None, op0=OP.subtract)
        qts_sb = const.tile([1, NQ], F32)
        nc.sync.dma_start(out=qts_sb[:], in_=qts)
        qts_bc = const.tile([P, NQ], F32)
        wr_sb = const.tile([1, D], F32)
        nc.sync.dma_start(out=wr_sb[:], in_=wr)
        wr_bc = const.tile([P, D], F32)
        with tc.tile_pool(name="ps_init", bufs=2, space="PSUM") as psi:
            t = psi.tile([P, NQ], F32, tag="qtsrep", name="qtsrep")
            nc.tensor.matmul(out=t[:], lhsT=ones_row[:], rhs=qts_sb[:],
                             start=True, stop=True)
            nc.vector.tensor_copy(qts_bc[:], t[:])
            for c in range(2):
                t2 = psi.tile([P, 512], F32, tag="wrrep", name="wrrep")
                nc.tensor.matmul(out=t2[:], lhsT=ones_row[:],
                                 rhs=wr_sb[:, 512 * c:512 * (c + 1)],
                                 start=True, stop=True)
                nc.vector.tensor_copy(wr_bc[:, 512 * c:512 * (c + 1)], t2[:])

        # causal masks, one per key tile kt, applied to the leading 128
        # query columns (slot s0 = kt//4): allow q_glob >= k_glob
        masks = [const.tile([P, P], BF16, tag=f"cmask{kt}", name=f"cmask{kt}")
                 for kt in range(NS)]
        thr = const.tile([P, 1], F32)
        for kt in range(NS):
            s0 = kt // 4
            nc.vector.tensor_scalar(thr[:], qts_bc[:, s0:s0 + 1], -128.0,
                                    float(P * kt), op0=OP.mult, op1=OP.add)
            nc.vector.tensor_scalar(masks[kt][:], fmp[:], thr[:], None,
                                    op0=OP.is_ge)

        # persistent small state
        s_all = const.tile([P, NT], F32)   # s_all[p, f] = score(token 128 f + p)
        idx_f = const.tile([P, NS], F32)   # sorted selected token ids (f32)
        idx_i = const.tile([P, NS], I32)
        myidx_i = const.tile([P, NQ], I32)

        # ---- early weight prefetch (wk; wq reuses the same slots later) ----
        wkq_pool = ctx.enter_context(tc.tile_pool(name="wkq", bufs=1))
        wk_t = [wkq_pool.tile([P, D], BF16, tag=f"wk{k}", name=f"wk{k}")
                for k in range(ND)]
        for k in range(ND):
            nc.scalar.dma_start(out=wk_t[k][:], in_=aps["wk"][P * k:P * (k + 1), :])

        # ---- phase 1: scores, streamed in 512-token chunks ----
        NCK = 8
        TPC = T // NCK          # 512 tokens per chunk
        FPC = TPC // P          # 4 f-columns per chunk
        with tc.tile_pool(name="score_x", bufs=3) as sxp, \
             tc.tile_pool(name="score_scr", bufs=2) as ssp:
            for ck in range(NCK):
                xt = sxp.tile([P, FPC * D], F32, tag="xt", name="xt")
                nc.sync.dma_start(
                    out=xt[:].rearrange("p (t d) -> p t d", d=D),
                    in_=x[TPC * ck:TPC * (ck + 1), :].rearrange(
                        "(t p) d -> p t d", p=P))
                scr = ssp.tile([P, FPC * D], F32, tag="scr", name="scr")
                for t in range(FPC):
                    nc.vector.tensor_tensor(
                        out=scr[:, D * t:D * (t + 1)],
                        in0=xt[:, D * t:D * (t + 1)], in1=wr_bc[:],
                        op=OP.mult)
                    nc.vector.tensor_reduce(
                        s_all[:, FPC * ck + t:FPC * ck + t + 1],
                        scr[:, D * t:D * (t + 1)], axis=AX.X, op=OP.add)

        # ---- phase 2: top-C threshold by 128-ary counting bisection ----
        cdf_ctx = ExitStack()
        with cdf_ctx:
            cdf = cdf_ctx.enter_context(tc.tile_pool(name="cdf", bufs=1))
            pss_ctx = cdf_ctx.enter_context(ExitStack())
            pss = pss_ctx.enter_context(
                tc.tile_pool(name="ps_s", bufs=2, space="PSUM"))
            sT_ps = pss.tile([NT, P], F32, tag="sT", name="sT")
            nc.tensor.transpose(out=sT_ps[:], in_=s_all[:], identity=ident_f[:])
            sT = cdf.tile([NT, P], F32)
            nc.vector.tensor_copy(sT[:], sT_ps[:])
            s_row = cdf.tile([1, T], F32)
            nc.sync.dma_start(
                out=s_row[0:1, :].rearrange("one (f p) -> one f p", p=P),
                in_=sT[:])
            s_rep = cdf.tile([P, T], F32)
            for c in range(T // 512):
                t = pss.tile([P, 512], F32, tag="srep", name="srep")
                nc.tensor.matmul(out=t[:], lhsT=ones_row[:],
                                 rhs=s_row[:, 512 * c:512 * (c + 1)],
                                 start=True, stop=True)
                nc.vector.tensor_copy(s_rep[:, 512 * c:512 * (c + 1)], t[:])
            lo = cdf.tile([P, 1], F32)
            nc.vector.memset(lo[:], -31.0)
            delta = cdf.tile([P, 1], F32)
            nc.vector.memset(delta[:], 64.0 / P)
            ge_scr = cdf.tile([P, T], F32)
            beta = cdf.tile([P, 1], F32)
            cnt = cdf.tile([P, 1], F32)
            mcnt = cdf.tile([P, 1], F32)
            nsel = cdf.tile([P, 1], F32)
            tmp1 = cdf.tile([P, 1], F32)
            for lev in range(5):
                nc.vector.tensor_tensor(out=beta[:], in0=iota_p1[:], in1=delta[:],
                                        op=OP.mult)
                nc.vector.tensor_tensor(out=beta[:], in0=beta[:], in1=lo[:],
                                        op=OP.add)
                nc.vector.tensor_scalar(ge_scr[:], s_rep[:], beta[:], 0.0,
                                        op0=OP.is_ge, op1=OP.add,
                                        accum_out=cnt[:])
                nc.vector.tensor_scalar(mcnt[:], cnt[:], float(C), None,
                                        op0=OP.is_ge)
                t = pss.tile([P, 1], F32, tag="nsel", name="nsel")
                nc.tensor.matmul(out=t[:], lhsT=ones_pp[:], rhs=mcnt[:],
                                 start=True, stop=True)
                nc.vector.tensor_copy(nsel[:], t[:])
                nc.vector.tensor_tensor(out=tmp1[:], in0=nsel[:], in1=delta[:],
                                        op=OP.mult)
                nc.vector.tensor_tensor(out=lo[:], in0=lo[:], in1=tmp1[:],
                                        op=OP.add)
                nc.vector.tensor_scalar_mul(delta[:], delta[:], 1.0 / P)

            # ---- phase 3: ranks + on-chip splice into sorted idx ----
            pss_ctx.close()
            pss2 = cdf_ctx.enter_context(
                tc.tile_pool(name="ps_r", bufs=1, space="PSUM"))
            msk = cdf.tile([P, NT], F32)
            nc.vector.tensor_scalar(msk[:], s_all[:], lo[:], None, op0=OP.is_ge)
            cs = cdf.tile([P, NT], F32)
            t = pss2.tile([P, NT], F32, tag="cs", name="cs")
            nc.tensor.matmul(out=t[:], lhsT=ones_pp[:], rhs=msk[:],
                             start=True, stop=True)
            nc.vector.tensor_copy(cs[:], t[:])
            pfa = cdf.tile([P, NT], F32)
            pfb = cdf.tile([P, NT], F32)
            nc.vector.tensor_copy(pfa[:], cs[:])
            cur, nxt = pfa, pfb
            sh = 1
            while sh < NT:
                nc.vector.tensor_copy(nxt[:, :sh], cur[:, :sh])
                nc.vector.tensor_tensor(out=nxt[:, sh:], in0=cur[:, sh:],
                                        in1=cur[:, :NT - sh], op=OP.add)
                cur, nxt = nxt, cur
                sh *= 2
            rank = cdf.tile([P, NT], F32)
            nc.vector.tensor_tensor(out=rank[:], in0=cur[:], in1=cs[:],
                                    op=OP.subtract)
            wrk = pss2.tile([P, NT], F32, tag="cs", name="cs")
            nc.tensor.matmul(out=wrk[:], lhsT=tri[:], rhs=msk[:],
                             start=True, stop=True)
            nc.vector.tensor_tensor(out=rank[:], in0=rank[:], in1=wrk[:],
                                    op=OP.add)
            # rdiv = rank div 128 (sum of threshold compares; rank <= 2048),
            # rmod = rank - 128 * rdiv
            rdiv = cdf.tile([P, NT], F32)
            rthr = cdf.tile([P, NT], F32)
            nc.vector.tensor_scalar(rdiv[:], rank[:], float(P), None,
                                    op0=OP.is_ge)
            for jthr in range(2, NS + 1):
                nc.vector.tensor_scalar(rthr[:], rank[:], float(P * jthr), None,
                                        op0=OP.is_ge)
                nc.vector.tensor_tensor(out=rdiv[:], in0=rdiv[:], in1=rthr[:],
                                        op=OP.add)
            rmod = cdf.tile([P, NT], F32)
            nc.vector.tensor_scalar(rmod[:], rdiv[:], -128.0, None, op0=OP.mult)
            nc.vector.tensor_tensor(out=rmod[:], in0=rmod[:], in1=rank[:],
                                    op=OP.add)
            # placement matmuls: idx_ps[p', j] = sum_f sum_p Comb_f[p,p'] rhs_f[p,j]
            idx_ps = pss2.tile([P, NS], F32, tag="idxps", name="idxps")
            with tc.tile_pool(name="splice", bufs=4) as spl:
                for f in range(NT):
                    comb = spl.tile([P, P], F32, tag="comb", name="comb")
                    nc.vector.tensor_scalar(comb[:], iota_fp[:],
                                            rmod[:, f:f + 1], None,
                                            op0=OP.is_equal)
                    nc.vector.tensor_scalar(comb[:], comb[:], msk[:, f:f + 1],
                                            None, op0=OP.mult)
                    jm = spl.tile([P, NS], F32, tag="jm", name="jm")
                    nc.vector.tensor_scalar(jm[:], iota16[:],
                                            rdiv[:, f:f + 1], None,
                                            op0=OP.is_equal)
                    nc.vector.tensor_scalar(jm[:], jm[:],
                                            iota_tok_f[:, f:f + 1], None,
                                            op0=OP.mult)
                    nc.tensor.matmul(out=idx_ps[:], lhsT=comb[:], rhs=jm[:],
                                     start=(f == 0), stop=(f == NT - 1))
            nc.vector.tensor_copy(idx_f[:], idx_ps[:])
            nc.vector.tensor_copy(idx_i[:], idx_f[:])
            nc.sync.dma_start(
                out=idx_out.rearrange("(f p) one -> p (f one)", p=P),
                in_=idx_i[:])

            # ---- phase 4: own-row indices via transpose + one-hot matmul ----
            idxT_ps = pss2.tile([NS, P], F32, tag="idxT", name="idxT")
            nc.tensor.transpose(out=idxT_ps[:], in_=idx_f[:], identity=ident_f[:])
            idxT = cdf.tile([NS, P], F32)
            nc.vector.tensor_copy(idxT[:], idxT_ps[:])
            oh = cdf.tile([NS, NQ], F32)
            for s in range(NQ):
                nc.vector.tensor_scalar(oh[:, s:s + 1], iota_p[0:NS, :],
                                        qts_bc[0:NS, s:s + 1], None,
                                        op0=OP.is_equal)
            my_ps = pss2.tile([P, NQ], F32, tag="myps", name="myps")
            nc.tensor.matmul(out=my_ps[:], lhsT=idxT[:], rhs=oh[:],
                             start=True, stop=True)
            myidx_f = cdf.tile([P, NQ], F32)
            nc.vector.tensor_copy(myidx_f[:], my_ps[:])
            nc.vector.tensor_copy(myidx_i[:], myidx_f[:])

        def rms_tile(pool, src, dst_bf):
            scr = pool.tile([P, 512], F32, tag="rms_scr", name="rms_scr")
            scr2 = pool.tile([P, 512], F32, tag="rms_scr2", name="rms_scr2")
            ssq = pool.tile([P, 1], F32, tag="rms_ssq", name="rms_ssq")
            ssq2 = pool.tile([P, 1], F32, tag="rms_ssq2", name="rms_ssq2")
            nc.scalar.activation(scr[:], src[:, 0:512], AF.Square,
                                 accum_out=ssq[:])
            nc.scalar.activation(scr2[:], src[:, 512:D], AF.Square,
                                 accum_out=ssq2[:])
            nc.vector.tensor_tensor(out=ssq[:], in0=ssq[:], in1=ssq2[:],
                                    op=OP.add)
            nc.vector.tensor_scalar(ssq[:], ssq[:], 1.0 / D, 1e-6,
                                    op0=OP.mult, op1=OP.add)
            nc.scalar.sqrt(ssq[:], ssq[:])
            nc.vector.reciprocal(ssq[:], ssq[:])
            nc.vector.tensor_scalar(dst_bf[:], src[:], ssq[:], None, op0=OP.mult)

        sel_pool = ctx.enter_context(tc.tile_pool(name="selown", bufs=1))
        sel_own = [sel_pool.tile([P, D], F32, tag=f"selown{q}", name=f"selown{q}")
                   for q in range(NQ)]

        with ExitStack() as attn_ctx:
            ao_pool = attn_ctx.enter_context(tc.tile_pool(name="aop", bufs=1))
            aoT = [ao_pool.tile([P, QCH], BF16, tag=f"aoT{d}", name=f"aoT{d}")
                   for d in range(ND)]
            kv_ctx = attn_ctx.enter_context(ExitStack())
            kv_pool = kv_ctx.enter_context(tc.tile_pool(name="kvp", bufs=1))
            KT = [kv_pool.tile([P, C], BF16, tag=f"KT{m}", name=f"KT{m}")
                  for m in range(ND)]
            VE = [kv_pool.tile([P, H * (HD + 1)], BF16, tag=f"VE{t}",
                               name=f"VE{t}") for t in range(NS)]
            QT = [kv_pool.tile([P, QCH], BF16, tag=f"QT{m}", name=f"QT{m}")
                  for m in range(ND)]
            xn_ctx = kv_ctx.enter_context(ExitStack())
            xn_pool = xn_ctx.enter_context(tc.tile_pool(name="xnp", bufs=1))
            xnT = [xn_pool.tile([P, C], BF16, tag=f"xnT{d}", name=f"xnT{d}")
                   for d in range(ND)]
            xnoT = [xn_pool.tile([P, QCH], BF16, tag=f"xnoT{d}", name=f"xnoT{d}")
                    for d in range(ND)]

            # ---- phase 5: gather + rmsnorm + transpose ----
            with tc.tile_pool(name="gth", bufs=2) as gth, \
                 tc.tile_pool(name="rmsp", bufs=2) as rmsp, \
                 tc.tile_pool(name="ps_tr", bufs=4, space="PSUM") as pstr:
                for j in range(NS):
                    selt = gth.tile([P, D], F32, tag="sel", name="sel")
                    nc.gpsimd.indirect_dma_start(
                        out=selt[:], out_offset=None, in_=x,
                        in_offset=bass.IndirectOffsetOnAxis(
                            ap=idx_i[:, j:j + 1], axis=0))
                    xnt = gth.tile([P, D], BF16, tag="xn", name="xn")
                    rms_tile(rmsp, selt, xnt)
                    for d in range(ND):
                        tp = pstr.tile([P, P], BF16, tag="tr", name="tr")
                        nc.tensor.transpose(out=tp[:], in_=xnt[:, P * d:P * (d + 1)],
                                            identity=ident_b[:])
                        nc.vector.tensor_copy(xnT[d][:, P * j:P * (j + 1)], tp[:])
                for q in range(NQ):
                    nc.gpsimd.indirect_dma_start(
                        out=sel_own[q][:], out_offset=None, in_=x,
                        in_offset=bass.IndirectOffsetOnAxis(
                            ap=myidx_i[:, q:q + 1], axis=0))
                    xnt = gth.tile([P, D], BF16, tag="xn", name="xn")
                    rms_tile(rmsp, sel_own[q], xnt)
                    for d in range(ND):
                        tp = pstr.tile([P, P], BF16, tag="tr", name="tr")
                        nc.tensor.transpose(out=tp[:], in_=xnt[:, P * d:P * (d + 1)],
                                            identity=ident_b[:])
                        nc.vector.tensor_copy(xnoT[d][:, P * q:P * (q + 1)], tp[:])

            # ---- phase 6: K^T (full), V(+ones), Q^T (own) ----
            with tc.tile_pool(name="ps_qkv", bufs=4, space="PSUM") as psq:
                for m in range(ND):
                    for n in range(C // 512):
                        ps = psq.tile([P, 512], F32, tag="qkv", name="qkv")
                        for k in range(ND):
                            nc.tensor.matmul(
                                out=ps[:], lhsT=wk_t[k][:, P * m:P * (m + 1)],
                                rhs=xnT[k][:, 512 * n:512 * (n + 1)],
                                start=(k == 0), stop=(k == ND - 1))
                        nc.scalar.copy(KT[m][:, 512 * n:512 * (n + 1)], ps[:])
                # wq reuses the wk slots; its DMA overlaps the V matmuls
                wq_t = [wkq_pool.tile([P, D], BF16, tag=f"wk{k}", name=f"wq{k}")
                        for k in range(ND)]
                for k in range(ND):
                    nc.scalar.dma_start(out=wq_t[k][:],
                                        in_=aps["wq"][P * k:P * (k + 1), :])
                with tc.tile_pool(name="wvp", bufs=1) as wvp:
                    wv_t = [wvp.tile([P, D], BF16, tag=f"wv{k}", name=f"wv{k}")
                            for k in range(ND)]
                    for k in range(ND):
                        nc.scalar.dma_start(out=wv_t[k][:],
                                            in_=aps["wv"][P * k:P * (k + 1), :])
                    for t in range(NS):
                        for n2 in range(2):
                            ps = psq.tile([P, 512], F32, tag="qkv", name="qkv")
                            for k in range(ND):
                                nc.tensor.matmul(
                                    out=ps[:], lhsT=xnT[k][:, P * t:P * (t + 1)],
                                    rhs=wv_t[k][:, 512 * n2:512 * (n2 + 1)],
                                    start=(k == 0), stop=(k == ND - 1))
                            dst = VE[t][:, (HD + 1) * 8 * n2:(HD + 1) * 8 * (n2 + 1)] \
                                .rearrange("p (h e) -> p h e", h=8)[:, :, 0:HD]
                            nc.vector.tensor_copy(
                                dst, ps[:].rearrange("p (h e) -> p h e", h=8))
                        ones_dst = VE[t][:].rearrange("p (h e) -> p h e",
                                                      h=H)[:, :, HD:HD + 1]
                        nc.vector.memset(ones_dst, 1.0)
                for m in range(ND):
                    ps = psq.tile([P, 512], F32, tag="qkv", name="qkv")
                    for k in range(ND):
                        nc.tensor.matmul(
                            out=ps[:], lhsT=wq_t[k][:, P * m:P * (m + 1)],
                            rhs=xnoT[k][:], start=(k == 0), stop=(k == ND - 1))
                    nc.scalar.copy(QT[m][:], ps[:])

            xn_ctx.close()

            # ---- phase 7: attention over the slot schedule ----
            # key tile kt serves query slots s >= kt//4 (columns 128*(kt//4)..512)
            with tc.tile_pool(name="pknl", bufs=6) as pknl, \
                 tc.tile_pool(name="ps_st", bufs=3, space="PSUM") as psst, \
                 tc.tile_pool(name="ps_av", bufs=1, space="PSUM") as psav, \
                 tc.tile_pool(name="ps_rep", bufs=1, space="PSUM") as psrep, \
                 tc.tile_pool(name="attn_sc", bufs=4) as attsc:
                for h in range(H):
                    m, r0 = h // 2, (h % 2) * HD
                    avt = [psav.tile([HD + 1, P], F32, tag=f"avt{s}",
                                     name=f"avt{s}") for s in range(NQ)]
                    for kt in range(NS):
                        c0 = P * (kt // 4)      # first query column served
                        w = QCH - c0
                        st = psst.tile([P, QCH], F32, tag="st", name="st")
                        nc.tensor.matmul(
                            out=st[:, 0:w],
                            lhsT=KT[m][r0:r0 + HD, P * kt:P * (kt + 1)],
                            rhs=QT[m][r0:r0 + HD, c0:QCH],
                            start=True, stop=True)
                        pt = pknl.tile([P, QCH], BF16, tag="pk", name="pk")
                        nc.scalar.activation(pt[:, 0:w], st[:, 0:w], AF.Exp,
                                             scale=0.125)
                        nc.vector.tensor_tensor(out=pt[:, 0:P], in0=pt[:, 0:P],
                                                in1=masks[kt][:], op=OP.mult)
                        for s in range(kt // 4, NQ):
                            nc.tensor.matmul(
                                out=avt[s][:],
                                lhsT=VE[kt][:, (HD + 1) * h:(HD + 1) * (h + 1)],
                                rhs=pt[:, P * s - c0:P * s - c0 + P],
                                start=(kt == 0), stop=(kt == 4 * s + 3))
                    av_sb = attsc.tile([HD + 1, QCH], F32, tag="avsb",
                                       name="avsb")
                    for s in range(NQ):
                        nc.vector.tensor_copy(av_sb[:, P * s:P * (s + 1)],
                                              avt[s][:])
                    rcp_row = attsc.tile([1, QCH], F32, tag="rcprow",
                                         name="rcprow")
                    nc.vector.reciprocal(rcp_row[:], av_sb[HD:HD + 1, :])
                    rep = psrep.tile([HD, QCH], F32, tag="rep", name="rep")
                    nc.tensor.matmul(out=rep[:], lhsT=ones_row[0:1, 0:HD],
                                     rhs=rcp_row[:], start=True, stop=True)
                    rcp = attsc.tile([HD, QCH], F32, tag="rcp", name="rcp")
                    nc.vector.tensor_copy(rcp[:], rep[:])
                    nc.vector.tensor_tensor(
                        out=aoT[h // 2][(h % 2) * HD:(h % 2) * HD + HD, :],
                        in0=av_sb[0:HD, :], in1=rcp[:], op=OP.mult)

            kv_ctx.close()

            # ---- phase 8: wo + residual + rmsnorm -> z^T ----
            r_pool = attn_ctx.enter_context(tc.tile_pool(name="rown", bufs=1))
            r_own = [r_pool.tile([P, D], F32, tag=f"r{q}", name=f"r{q}")
                     for q in range(NQ)]
            z_pool = attn_ctx.enter_context(tc.tile_pool(name="zp", bufs=1))
            zT = [z_pool.tile([P, QCH], BF16, tag=f"zT{d}", name=f"zT{d}")
                  for d in range(ND)]
            with tc.tile_pool(name="wop", bufs=1) as wop, \
                 tc.tile_pool(name="rms2", bufs=2) as rms2, \
                 tc.tile_pool(name="ps_wo", bufs=4, space="PSUM") as pswo:
                wo_t = [wop.tile([P, D], BF16, tag=f"wo{k}", name=f"wo{k}")
                        for k in range(ND)]
                for k in range(ND):
                    nc.scalar.dma_start(out=wo_t[k][:],
                                        in_=aps["wo"][P * k:P * (k + 1), :])
                for q in range(NQ):
                    for n2 in range(2):
                        ps = pswo.tile([P, 512], F32, tag="wo_ps", name="wo_ps")
                        for k in range(ND):
                            nc.tensor.matmul(
                                out=ps[:], lhsT=aoT[k][:, P * q:P * (q + 1)],
                                rhs=wo_t[k][:, 512 * n2:512 * (n2 + 1)],
                                start=(k == 0), stop=(k == ND - 1))
                        nc.vector.tensor_tensor(
                            out=r_own[q][:, 512 * n2:512 * (n2 + 1)],
                            in0=sel_own[q][:, 512 * n2:512 * (n2 + 1)],
                            in1=ps[:], op=OP.add)
                    zt = rms2.tile([P, D], BF16, tag="z", name="z")
                    rms_tile(rms2, r_own[q], zt)
                    for d in range(ND):
                        tp = pswo.tile([P, P], BF16, tag="tr2", name="tr2")
                        nc.tensor.transpose(out=tp[:], in_=zt[:, P * d:P * (d + 1)],
                                            identity=ident_b[:])
                        nc.vector.tensor_copy(zT[d][:, P * q:P * (q + 1)], tp[:])

            # ---- phase 9: gate/up/silu -> X^T ----
            xt_pool = attn_ctx.enter_context(tc.tile_pool(name="xtp", bufs=1))
            XT = [xt_pool.tile([P, QCH], BF16, tag=f"XT{f}", name=f"XT{f}")
                  for f in range(NFF)]
            with tc.tile_pool(name="wgu", bufs=2) as wgup, \
                 tc.tile_pool(name="gu_sc", bufs=4) as gusc, \
                 tc.tile_pool(name="ps_gu", bufs=4, space="PSUM") as psgu:
                for fb in range(FF // 512):
                    wg_b = [wgup.tile([P, 512], BF16, tag=f"wg{k}", name=f"wg{k}")
                            for k in range(ND)]
                    wu_b = [wgup.tile([P, 512], BF16, tag=f"wu{k}", name=f"wu{k}")
                            for k in range(ND)]
                    for k in range(ND):
                        nc.sync.dma_start(
                            out=wg_b[k][:],
                            in_=aps["wg"][P * k:P * (k + 1), 512 * fb:512 * (fb + 1)])
                        nc.scalar.dma_start(
                            out=wu_b[k][:],
                            in_=aps["wu"][P * k:P * (k + 1), 512 * fb:512 * (fb + 1)])
                    for fm in range(4):
                        gps = psgu.tile([P, 512], F32, tag="g", name="g")
                        ups = psgu.tile([P, 512], F32, tag="u", name="u")
                        for k in range(ND):
                            nc.tensor.matmul(
                                out=gps[:], lhsT=wg_b[k][:, P * fm:P * (fm + 1)],
                                rhs=zT[k][:], start=(k == 0), stop=(k == ND - 1))
                        for k in range(ND):
                            nc.tensor.matmul(
                                out=ups[:], lhsT=wu_b[k][:, P * fm:P * (fm + 1)],
                                rhs=zT[k][:], start=(k == 0), stop=(k == ND - 1))
                        gs = gusc.tile([P, 512], BF16, tag="gs", name="gs")
                        nc.scalar.activation(gs[:], gps[:], AF.Sigmoid)
                        gu = gusc.tile([P, 512], BF16, tag="gu", name="gu")
                        nc.vector.tensor_tensor(out=gu[:], in0=gs[:], in1=gps[:],
                                                op=OP.mult)
                        nc.vector.tensor_tensor(out=XT[4 * fb + fm][:], in0=gu[:],
                                                in1=ups[:], op=OP.mult)

            # ---- phase 10: down projection + residual -> y ----
            with tc.tile_pool(name="wdp", bufs=3) as wdp, \
                 tc.tile_pool(name="yp", bufs=2) as yp, \
                 tc.tile_pool(name="ps_mlp", bufs=1, space="PSUM") as psml:
                mlp_ps = [psml.tile([P, 512], F32, tag=f"mlp{i}", name=f"mlp{i}")
                          for i in range(2 * NQ)]
                for f in range(NFF):
                    wd_t = wdp.tile([P, D], BF16, tag="wd", name="wd")
                    nc.sync.dma_start(out=wd_t[:],
                                      in_=aps["wd"][P * f:P * (f + 1), :])
                    for q in range(NQ):
                        for n2 in range(2):
                            nc.tensor.matmul(
                                out=mlp_ps[2 * q + n2][:],
                                lhsT=XT[f][:, P * q:P * (q + 1)],
                                rhs=wd_t[:, 512 * n2:512 * (n2 + 1)],
                                start=(f == 0), stop=(f == NFF - 1))
                for q in range(NQ):
                    yt = yp.tile([P, D], F32, tag="y", name="y")
                    for n2 in range(2):
                        nc.vector.tensor_tensor(
                            out=yt[:, 512 * n2:512 * (n2 + 1)],
                            in0=r_own[q][:, 512 * n2:512 * (n2 + 1)],
                            in1=mlp_ps[2 * q + n2][:], op=OP.add)
                    nc.sync.dma_start(out=y_out[P * q:P * (q + 1), :], in_=yt[:])


_NC_CACHE = {}


def _get_nc():
    if "nc" not in _NC_CACHE:
        _NC_CACHE["nc"] = build_nc()
    return _NC_CACHE["nc"]


def make_in_maps(inputs):
    bf = ml_dtypes.bfloat16
    x = np.asarray(inputs["x"], np.float32)
    g1 = np.asarray(inputs["g1"], np.float32).reshape(D, 1)
    g2 = np.asarray(inputs["g2"], np.float32).reshape(D, 1)
    wq_ = (g1 * np.asarray(inputs["wq"], np.float32)).astype(bf)
    wk_ = (g1 * np.asarray(inputs["wk"], np.float32)).astype(bf)
    wv_ = (g1 * np.asarray(inputs["wv"], np.float32)).astype(bf)
    wo_ = np.asarray(inputs["wo"], np.float32).astype(bf)
    wg_ = (g2 * np.asarray(inputs["w_gate"], np.float32)).astype(bf)
    wu_ = (g2 * np.asarray(inputs["w_up"], np.float32)).astype(bf)
    wd_ = np.asarray(inputs["w_down"], np.float32).astype(bf)
    wr_ = np.asarray(inputs["w_router"], np.float32).reshape(1, D)
    in_maps = []
    for c in range(N_CORES):
        g, j = c // GROUP, c % GROUP
        in_maps.append({
            "x": np.ascontiguousarray(x[g]),
            "wr": wr_,
            "qts": np.array([slot_tiles(j)], np.float32),
            "wq": wq_, "wk": wk_, "wv": wv_, "wo": wo_,
            "wg": wg_, "wu": wu_, "wd": wd_,
        })
    return in_maps


def assemble(inputs, results):
    x = np.asarray(inputs["x"], np.float32)
    out = x.copy()
    for g in range(B):
        idx = results[GROUP * g]["idx"][:, 0]
        for j in range(GROUP):
            y = results[GROUP * g + j]["y"]
            for s, qt in enumerate(slot_tiles(j)):
                out[g, idx[P * qt:P * (qt + 1)]] = y[P * s:P * (s + 1)]
    return out


def kernel(**inputs):
    nc = _get_nc()
    res = run_bass_kernel_spmd(nc, make_in_maps(inputs),
                               core_ids=list(range(N_CORES)))
    return assemble(inputs, res.results)

